# revision 1
# baseline (speedup 1.0000x reference)
"""GCN (3-layer, PyG GCNConv semantics) on 8 Trainium2 NeuronCores.

Strategy (per sharding hint):
  - Nodes are packed into 8*NBLK blocks of 128 slots each, degree-balanced
    (greedy bin packing by in-degree) so every block has ~equal edge count.
    Core c owns blocks [c*NBLK, (c+1)*NBLK).
  - Edges are partitioned by destination block; each block's edge list is
    padded to T tiles of 128 edges (dummy edges get rel_dst=255 -> zero
    one-hot column).
  - Aggregation per dst block: per 128-edge tile, a one-hot selection matrix
    (DVE iota == rel_dst) right-multiplies the [edge, feat] tile on the PE,
    accumulating [feat, dst] in PSUM (transposed so the next layer's
    transform consumes h^T straight from SBUF). dinv[dst] and bias+relu are
    applied at PSUM flush; dinv[src] is folded into the feature tables.
  - Layer 1 needs no gathers: the host pre-gathers x into edge-slot order
    (xgT, part of sharding prep) and the kernel transforms each edge tile
    with W1 on the fly.
  - Layers 2-3: transform h^T locally, AllGather the bf16 table, then fetch
    each edge tile's source rows with a per-partition indirect DMA (the only
    gather primitive this stack supports: 128 row-descriptors per call).
  - Final layer aggregates in [dst, class] layout and applies log-softmax
    on-device before writing the output shard.

Sync-legality: this neuronxcc build allows at most ONE semaphore wait per
instruction. Tile emits minimal waits via per-engine vector clocks; a
post-pass (_legalize_waits) spills excess waits onto same-engine NoOps.
Absorber nops/pre-touches keep the hot loops at <=1 natural wait.
"""

import math

import numpy as np
import ml_dtypes

BF16 = ml_dtypes.bfloat16

# hardcoded problem shape (nn_GCNModel_68186900792261)
N = 50000
F_IN = 128
HID = 128
C = 40
NCORES = 8
P = 128
NBLK = 49          # blocks of 128 node slots per core
NPAD = NBLK * P    # padded nodes per core
NPADT = NCORES * NPAD
NB_GROUP = 4       # dst blocks per gather/sel batch
WG = 8             # transform tiles per batched write


def _pack_nodes(deg):
    """Greedy degree-balanced packing of nodes into NCORES*NBLK bins of <=P."""
    import heapq

    nbins = NCORES * NBLK
    order = np.argsort(-deg, kind="stable")
    heap = [(0, b) for b in range(nbins)]
    heapq.heapify(heap)
    nodecnt = np.zeros(nbins, np.int64)
    pos_of_node = np.empty(len(deg), np.int64)
    for n in order:
        while True:
            e, b = heapq.heappop(heap)
            if nodecnt[b] < P:
                break
        core, blk = divmod(b, NBLK)
        pos_of_node[n] = core * NPAD + blk * P + nodecnt[b]
        nodecnt[b] += 1
        heapq.heappush(heap, (e + int(deg[n]), b))
    return pos_of_node


def _preprocess(x, W1, b1, W2, b2, W3, b3, edge_index):
    src = np.asarray(edge_index[0], dtype=np.int64)
    dst = np.asarray(edge_index[1], dtype=np.int64)
    loop = np.arange(N, dtype=np.int64)
    src_all = np.concatenate([src, loop])
    dst_all = np.concatenate([dst, loop])

    deg = np.bincount(dst_all, minlength=N).astype(np.float64)
    dinv = 1.0 / np.sqrt(deg)

    pos_of_node = _pack_nodes(deg)
    binidx = pos_of_node // P
    slot = pos_of_node % P

    ebin = binidx[dst_all]
    order = np.argsort(ebin, kind="stable")
    ebin_s = ebin[order]
    counts = np.bincount(ebin_s, minlength=NCORES * NBLK)
    T = int(math.ceil(counts.max() / P))
    cap = T * P
    offs = np.zeros(NCORES * NBLK, np.int64)
    offs[1:] = np.cumsum(counts)[:-1]
    rank = np.arange(len(ebin_s)) - offs[ebin_s]

    big_src = np.zeros((NCORES * NBLK, cap), np.int32)
    big_rel = np.full((NCORES * NBLK, cap), 255.0, np.float32)
    big_src[ebin_s, rank] = pos_of_node[src_all[order]].astype(np.int32)
    big_rel[ebin_s, rank] = slot[dst_all[order]].astype(np.float32)

    GT = NBLK * T
    src_idx = (
        big_src.reshape(NCORES, NBLK, T, P).transpose(0, 3, 1, 2).reshape(NCORES, P, GT)
    )
    relseg = (
        big_rel.reshape(NCORES, NBLK, T, P)
        .transpose(0, 3, 1, 2)
        .reshape(NCORES, P, GT)
        .astype(BF16)
    )

    dinv_pos = np.zeros(NPADT, np.float32)
    dinv_pos[pos_of_node] = dinv.astype(np.float32)
    dinv_cols = dinv_pos.reshape(NCORES * NBLK, P).T.copy()
    dinv_rows = dinv_pos.reshape(NCORES, 1, NPAD)

    # layer-1 inputs pre-gathered in edge-slot order, dinv[src] folded in
    xpos = np.zeros((NPADT, F_IN), np.float32)
    xpos[pos_of_node] = np.asarray(x, np.float32) * dinv.astype(np.float32)[:, None]

    iota = np.tile(np.arange(P, dtype=np.float32), (P, 1)).astype(BF16)

    common = {
        "W1": np.asarray(W1, np.float32).astype(BF16),
        "W2": np.asarray(W2, np.float32).astype(BF16),
        "W3": np.asarray(W3, np.float32).astype(BF16),
        "b1": np.asarray(b1, np.float32).reshape(P, 1),
        "b2": np.asarray(b2, np.float32).reshape(P, 1),
        "b3": np.tile(np.asarray(b3, np.float32).reshape(1, C), (P, 1)),
        "iota": iota,
    }
    in_maps = []
    for c in range(NCORES):
        m = dict(common)
        sidx = src_idx[c]                          # [P, GT]
        m["src_idx"] = np.ascontiguousarray(sidx)
        m["relseg"] = np.ascontiguousarray(relseg[c])
        m["dinv_bc"] = np.tile(dinv_rows[c], (P, 1))
        m["dinv_cols_loc"] = np.ascontiguousarray(
            dinv_cols[:, c * NBLK : (c + 1) * NBLK]
        )
        # xgsw[p, g*F + f] = xpos[src_idx[p, g], f] (edge-major tiles)
        xg = xpos[sidx]                            # [P, GT, F]
        m["xgsw"] = np.ascontiguousarray(xg.reshape(P, GT * F_IN)).astype(BF16)
        in_maps.append(m)
    return in_maps, pos_of_node, T


def _legalize_waits(nc, mybir, max_waits=1):
    """This neuronxcc build allows at most one sem wait per instruction.

    Spill excess waits onto same-engine NoOps inserted immediately before the
    offending instruction (the engine sequencer executes them in order, so
    semantics are identical).
    """
    wn = 0
    for func in nc.m.functions:
        for bb in func.blocks:
            out = []
            changed = False
            for ins in bb.instructions:
                si = ins.sync_info
                if si is not None and si.on_wait and len(si.on_wait) > max_waits:
                    waits = list(si.on_wait)
                    for w in waits[:-max_waits]:
                        nop = mybir.InstNoOp(
                            name=f"WSPILL-{wn}",
                            engine=ins.engine,
                            sync_info=mybir.SyncInfo(on_wait=[w], on_update=[]),
                        )
                        wn += 1
                        out.append(nop)
                    ins.sync_info = mybir.SyncInfo(
                        on_wait=waits[-max_waits:], on_update=list(si.on_update)
                    )
                    changed = True
                out.append(ins)
            if changed:
                bb.instructions = out
    return wn


def _build_nc(T, legalize=True):
    import concourse.bass as bass
    import concourse.mybir as mybir
    import concourse.tile as tile
    from concourse.tile_rust import add_dep_helper

    f32 = mybir.dt.float32
    bf16 = mybir.dt.bfloat16
    i32 = mybir.dt.int32
    GT = NBLK * T
    SLOTS = GT * P

    nc = bass.Bass(target_bir_lowering=False, debug=False, num_devices=NCORES)

    xgsw = nc.dram_tensor("xgsw", [P, SLOTS], bf16, kind="ExternalInput")
    W1 = nc.dram_tensor("W1", [F_IN, HID], bf16, kind="ExternalInput")
    W2 = nc.dram_tensor("W2", [HID, HID], bf16, kind="ExternalInput")
    W3 = nc.dram_tensor("W3", [HID, C], bf16, kind="ExternalInput")
    b1 = nc.dram_tensor("b1", [P, 1], f32, kind="ExternalInput")
    b2 = nc.dram_tensor("b2", [P, 1], f32, kind="ExternalInput")
    b3 = nc.dram_tensor("b3", [P, C], f32, kind="ExternalInput")
    iota = nc.dram_tensor("iota", [P, P], bf16, kind="ExternalInput")
    dinv_cols_loc = nc.dram_tensor("dinv_cols_loc", [P, NBLK], f32, kind="ExternalInput")
    dinv_bc = nc.dram_tensor("dinv_bc", [P, NPAD], f32, kind="ExternalInput")
    src_idx = nc.dram_tensor("src_idx", [P, GT], i32, kind="ExternalInput")
    relseg = nc.dram_tensor("relseg", [P, GT], bf16, kind="ExternalInput")
    out = nc.dram_tensor("out", [NPAD, C], f32, kind="ExternalOutput")

    agin2 = nc.dram_tensor("agin2", [NPAD, HID], bf16)
    table2 = nc.dram_tensor("table2", [NPADT, HID], bf16, addr_space="Shared")
    agin3 = nc.dram_tensor("agin3", [NPAD, C], bf16)
    table3 = nc.dram_tensor("table3", [NPADT, C], bf16, addr_space="Shared")

    groups = [[i for i in range(NCORES)]]
    tail_deps = []

    def dep(later, earlier, sync=True, reason="gcn"):
        add_dep_helper(later.ins, earlier.ins, sync=sync, reason=reason)

    with tile.TileContext(nc) as tc:
        with (
            tc.tile_pool(name="const", bufs=1) as cpool,
            tc.tile_pool(name="hbuf", bufs=1) as hpool,
            tc.tile_pool(name="xload", bufs=2) as xpool,
            tc.tile_pool(name="mt", bufs=4) as mtpool,
            tc.tile_pool(name="gather", bufs=8) as gpool,
            tc.tile_pool(name="sel", bufs=2) as spool,
            tc.tile_pool(name="stw", bufs=4) as fpool,
            tc.tile_pool(name="aflush", bufs=4) as f2pool,
            tc.tile_pool(name="aggt", bufs=2) as agpool,
            tc.tile_pool(name="smax", bufs=4) as mpool,
            tc.tile_pool(name="pst", bufs=2, space="PSUM") as tpsum,
            tc.tile_pool(name="pst2", bufs=2, space="PSUM") as t2psum,
            tc.tile_pool(name="psa", bufs=4, space="PSUM") as apsum,
        ):
            sb_W1 = cpool.tile([F_IN, HID], bf16, tag="w1")
            nc.sync.dma_start(out=sb_W1[:], in_=W1[:, :])
            sb_W2 = cpool.tile([HID, HID], bf16, tag="w2")
            nc.sync.dma_start(out=sb_W2[:], in_=W2[:, :])
            sb_W3 = cpool.tile([HID, C], bf16, tag="w3")
            nc.sync.dma_start(out=sb_W3[:], in_=W3[:, :])
            sb_b1 = cpool.tile([P, 1], f32, tag="b1")
            nc.sync.dma_start(out=sb_b1[:], in_=b1[:, :])
            sb_b2 = cpool.tile([P, 1], f32, tag="b2")
            nc.sync.dma_start(out=sb_b2[:], in_=b2[:, :])
            sb_b3 = cpool.tile([P, C], f32, tag="b3")
            nc.sync.dma_start(out=sb_b3[:], in_=b3[:, :])
            sb_iota = cpool.tile([P, P], bf16, tag="iota")
            nc.sync.dma_start(out=sb_iota[:], in_=iota[:, :])
            sb_dcolsl = cpool.tile([P, NBLK], f32, tag="dcolsl")
            nc.sync.dma_start(out=sb_dcolsl[:], in_=dinv_cols_loc[:, :])
            sb_dbc = cpool.tile([P, NPAD], f32, tag="dbc")
            nc.sync.dma_start(out=sb_dbc[:], in_=dinv_bc[:, :])
            sb_idx = cpool.tile([P, GT], i32, tag="sidx")
            nc.sync.dma_start(out=sb_idx[:], in_=src_idx[:, :])
            sb_rel = cpool.tile([P, GT], bf16, tag="srel")
            nc.sync.dma_start(out=sb_rel[:], in_=relseg[:, :])

            # Pre-touch constants on their consuming engines.
            scr = cpool.tile([P, 8], f32, tag="scratch")
            nc.vector.tensor_copy(out=scr[:, 0:1], in_=sb_iota[:, 0:1])
            nc.vector.tensor_copy(out=scr[:, 1:2], in_=sb_rel[:, 0:1])
            nc.vector.tensor_copy(out=scr[:, 2:3], in_=sb_dbc[:, 0:1])
            nc.vector.tensor_copy(out=scr[:, 3:4], in_=sb_b3[:, 0:1])
            nc.vector.tensor_copy(out=scr[:, 4:5], in_=sb_dcolsl[:, 0:1])
            nc.scalar.activation(
                out=scr[:, 5:6], in_=sb_b1[:, 0:1],
                func=mybir.ActivationFunctionType.Copy,
            )
            nc.scalar.activation(
                out=scr[:, 6:7], in_=sb_b2[:, 0:1],
                func=mybir.ActivationFunctionType.Copy,
            )
            scr2 = cpool.tile([P, 2], i32, tag="scratch2")
            nc.gpsimd.tensor_copy(out=scr2[:, 0:1], in_=sb_idx[:, 0:1])
            nc.tensor.ldweights(weights=sb_W1[:])
            nc.tensor.ldweights(weights=sb_W2[:])
            nc.tensor.ldweights(weights=sb_W3[:])

            sb_h1T = hpool.tile([P, NPAD], bf16, tag="h1T")
            sb_h2T = hpool.tile([P, NPAD], bf16, tag="h2T")

            def build_sel(g0, nb):
                gcols = nb * T
                sel = spool.tile([P, NB_GROUP * T * P], bf16, tag="sel")
                sel3 = sel[:, : gcols * P].rearrange("p (g s) -> p g s", s=P)
                nc.vector.tensor_tensor(
                    out=sel3,
                    in0=sb_iota[:, None, :].to_broadcast([P, gcols, P]),
                    in1=sb_rel[:, g0 * T : g0 * T + gcols, None].to_broadcast(
                        [P, gcols, P]
                    ),
                    op=mybir.AluOpType.is_equal,
                )
                return sel

            def flush_block(ps, blk, outT_sb, bias_col):
                tmp = f2pool.tile([P, P], mybir.dt.float32, tag="at")
                nc.vector.tensor_tensor(
                    out=tmp[:],
                    in0=ps[:],
                    in1=sb_dbc[:, blk * P : (blk + 1) * P],
                    op=mybir.AluOpType.mult,
                )
                return nc.scalar.activation(
                    out=outT_sb[:, blk * P : (blk + 1) * P],
                    in_=tmp[:],
                    func=mybir.ActivationFunctionType.Relu,
                    bias=bias_col[:, :1],
                )

            # ---- Layer 1: scatter-first — aggregate x tiles, then W1 ----
            for g0 in range(0, NBLK, NB_GROUP):
                nb = min(NB_GROUP, NBLK - g0)
                gcols = nb * T
                xg = xpool.tile([P, NB_GROUP * T * P], bf16, tag="xg")
                nc.sync.dma_start(
                    out=xg[:, : gcols * P],
                    in_=xgsw[:, g0 * T * P : (g0 * T + gcols) * P],
                )
                sel = build_sel(g0, nb)
                aggT = agpool.tile([P, NB_GROUP * P], bf16, tag="aggT")
                for b in range(nb):
                    blk = g0 + b
                    ps = apsum.tile([P, P], mybir.dt.float32, tag="psa")
                    for t in range(T):
                        g = b * T + t
                        nc.tensor.matmul(
                            out=ps[:],
                            lhsT=xg[:, g * P : (g + 1) * P],
                            rhs=sel[:, g * P : (g + 1) * P],
                            start=(t == 0),
                            stop=(t == T - 1),
                        )
                    nc.vector.tensor_tensor(
                        out=aggT[:, b * P : (b + 1) * P],
                        in0=ps[:],
                        in1=sb_dbc[:, blk * P : (blk + 1) * P],
                        op=mybir.AluOpType.mult,
                    )
                ps2 = t2psum.tile([P, NB_GROUP * P], mybir.dt.float32, tag="pst2")
                nc.tensor.matmul(
                    out=ps2[:, : nb * P],
                    lhsT=sb_W1[:],
                    rhs=aggT[:, : nb * P],
                    start=True,
                    stop=True,
                )
                nc.scalar.activation(
                    out=sb_h1T[:, g0 * P : (g0 + nb) * P],
                    in_=ps2[:, : nb * P],
                    func=mybir.ActivationFunctionType.Relu,
                    bias=sb_b1[:, :1],
                )

            def transform(src_sb, W_sb, dinv_ap, dst_dram, width):
                """dst_dram = dinv * (h @ W) in bf16 (batched writes)."""
                write_dmas = []
                for g0 in range(0, NBLK, WG):
                    nb = min(WG, NBLK - g0)
                    stw = fpool.tile([P, WG * width], bf16, tag="tflush")
                    guard = None
                    if len(write_dmas) >= 4:
                        guard = nc.vector.nop()
                        dep(guard, write_dmas[-4], sync=True, reason="stw WAR")
                    for k in range(nb):
                        t = g0 + k
                        ps = tpsum.tile([P, width], mybir.dt.float32, tag="pst")
                        nc.tensor.matmul(
                            out=ps[:],
                            lhsT=src_sb[:, t * P : (t + 1) * P],
                            rhs=W_sb[:],
                            start=True,
                            stop=True,
                        )
                        fl = nc.vector.tensor_scalar(
                            out=stw[:, k * width : (k + 1) * width],
                            in0=ps[:],
                            scalar1=dinv_ap[:, t : t + 1],
                            scalar2=None,
                            op0=mybir.AluOpType.mult,
                        )
                        if guard is not None and k == 0:
                            dep(fl, guard, sync=False, reason="flush after absorb")
                    wd = nc.sync.dma_start(
                        out=dst_dram[g0 * P : (g0 + nb) * P, :].rearrange(
                            "(k p) f -> p k f", p=P
                        ),
                        in_=stw[:, : nb * width].rearrange("p (k f) -> p k f", f=width),
                    )
                    write_dmas.append(wd)
                return write_dmas

            def pool_absorb(dmas, reason):
                last = None
                for d in dmas:
                    n = nc.gpsimd.nop()
                    dep(n, d, sync=True, reason=reason)
                    last = n
                return last

            def aggregate(table, width, outT_sb, bias_col, after=None,
                          final_blocks=False):
                gathers = []
                last_insts = []
                for g0 in range(0, NBLK, NB_GROUP):
                    nb = min(NB_GROUP, NBLK - g0)
                    sel = build_sel(g0, nb)
                    for b in range(nb):
                        blk = g0 + b
                        ps = apsum.tile(
                            [P, P if not final_blocks else C],
                            mybir.dt.float32, tag="psa",
                        )
                        for t in range(T):
                            g = b * T + t
                            gt = gpool.tile([P, width], bf16, tag="gt")
                            g_ins = nc.gpsimd.indirect_dma_start(
                                out=gt[:],
                                out_offset=None,
                                in_=table[:, :],
                                in_offset=bass.IndirectOffsetOnAxis(
                                    ap=sb_idx[:, g0 * T + g : g0 * T + g + 1], axis=0
                                ),
                            )
                            if not gathers and after is not None:
                                dep(g_ins, after, sync=False, reason="gather order")
                            gathers.append(g_ins)
                            if not final_blocks:
                                mm = nc.tensor.matmul(
                                    out=ps[:],
                                    lhsT=gt[:],
                                    rhs=sel[:, g * P : (g + 1) * P],
                                    start=(t == 0),
                                    stop=(t == T - 1),
                                )
                            else:
                                mm = nc.tensor.matmul(
                                    out=ps[:],
                                    lhsT=sel[:, g * P : (g + 1) * P],
                                    rhs=gt[:],
                                    start=(t == 0),
                                    stop=(t == T - 1),
                                )
                        if not final_blocks:
                            fl = flush_block(ps, blk, outT_sb, bias_col)
                            last_insts = [mm, fl]
                        else:
                            t0 = mpool.tile([P, C], mybir.dt.float32, tag="t0")
                            nc.vector.tensor_scalar(
                                out=t0[:], in0=ps[:],
                                scalar1=sb_dcolsl[:, blk : blk + 1],
                                scalar2=None, op0=mybir.AluOpType.mult,
                            )
                            t1 = mpool.tile([P, C], mybir.dt.float32, tag="t1")
                            nc.vector.tensor_tensor(
                                out=t1[:], in0=t0[:], in1=sb_b3[:],
                                op=mybir.AluOpType.add,
                            )
                            mx = mpool.tile([P, 1], mybir.dt.float32, tag="mx")
                            nc.vector.tensor_reduce(
                                out=mx[:], in_=t1[:],
                                axis=mybir.AxisListType.X, op=mybir.AluOpType.max,
                            )
                            nmx = mpool.tile([P, 1], mybir.dt.float32, tag="nmx")
                            nc.vector.tensor_scalar(
                                out=nmx[:], in0=mx[:], scalar1=-1.0, scalar2=None,
                                op0=mybir.AluOpType.mult,
                            )
                            ex = mpool.tile([P, C], mybir.dt.float32, tag="ex")
                            ssum = mpool.tile([P, 1], mybir.dt.float32, tag="ssum")
                            nc.scalar.activation(
                                out=ex[:], in_=t1[:],
                                func=mybir.ActivationFunctionType.Exp,
                                bias=nmx[:, :1], accum_out=ssum[:, :1],
                            )
                            ls = mpool.tile([P, 1], mybir.dt.float32, tag="ls")
                            act2 = nc.scalar.activation(
                                out=ls[:], in_=ssum[:],
                                func=mybir.ActivationFunctionType.Ln,
                            )
                            tot = mpool.tile([P, 1], mybir.dt.float32, tag="tot")
                            nc.vector.tensor_tensor(
                                out=tot[:], in0=mx[:], in1=ls[:],
                                op=mybir.AluOpType.add,
                            )
                            fin = mpool.tile([P, C], mybir.dt.float32, tag="fin")
                            fts = nc.vector.tensor_scalar(
                                out=fin[:], in0=t1[:], scalar1=tot[:, :1],
                                scalar2=None, op0=mybir.AluOpType.subtract,
                            )
                            od = nc.sync.dma_start(
                                out=out[blk * P : (blk + 1) * P, :], in_=fin[:]
                            )
                            last_insts = [mm, fts, act2, od]
                            tail_deps.append(od)
                return gathers, last_insts

            # ---- Layer 2: local transform + AllGather + aggregate ----
            wds2 = transform(sb_h1T, sb_W2, sb_dcolsl[:], agin2, HID)
            ab2 = pool_absorb(wds2, "agin2 writes")
            cc2 = nc.gpsimd.collective_compute(
                "AllGather",
                mybir.AluOpType.bypass,
                replica_groups=groups,
                ins=[agin2.ap().opt()],
                outs=[table2.ap().opt()],
            )
            dep(cc2, ab2, sync=False, reason="cc after absorb")
            aggregate(table2, HID, sb_h2T, sb_b2)

            # ---- Layer 3: local transform + AllGather + aggregate/softmax ----
            wds3 = transform(sb_h2T, sb_W3, sb_dcolsl[:], agin3, C)
            ab3 = pool_absorb(wds3, "agin3 writes")
            cc3 = nc.gpsimd.collective_compute(
                "AllGather",
                mybir.AluOpType.bypass,
                replica_groups=groups,
                ins=[agin3.ap().opt()],
                outs=[table3.ap().opt()],
            )
            dep(cc3, ab3, sync=False, reason="cc after absorb")
            g3, last3 = aggregate(table3, C, None, None, final_blocks=True)

            # ---- tail quiesce ----
            for d in tail_deps + last3 + [cc2, cc3] + g3[-16:]:
                n = nc.sync.nop()
                dep(n, d, sync=True, reason="tail quiesce")

    if legalize:
        _legalize_waits(nc, mybir)
    return nc


def _run(inputs, trace=False):
    import sys

    if "/opt/trn_rl_repo" not in sys.path:
        sys.path.insert(0, "/opt/trn_rl_repo")
    from concourse.bass_utils import run_bass_kernel_spmd

    in_maps, pos_of_node, T = _preprocess(**inputs)
    nc = _build_nc(T)
    res = run_bass_kernel_spmd(
        nc, in_maps, core_ids=list(range(NCORES)), trace=trace
    )
    outs = np.concatenate([res.results[c]["out"] for c in range(NCORES)], axis=0)
    full = outs[pos_of_node].astype(np.float32)
    return full, res


def kernel(**inputs):
    full, _ = _run(inputs, trace=False)
    return full



# revision 11
# speedup vs baseline: 1.1684x; 1.1684x over previous
"""GCN (3-layer, PyG GCNConv semantics) on 8 Trainium2 NeuronCores.

Strategy v2:
  - Nodes packed into 8*NBLK blocks of 128 slots, degree-balanced.
  - Conv is linear, so every layer aggregates FIRST (A @ table) and
    transforms after, per dst block:
        psum_agg[feat, dst]  = sum_tiles gathered_tile @ onehot_sel
        psum_tf [dst, fout]  = matmul(lhsT=aggT, rhs=W)
        table_next[dst, :]   = dinv * relu(dinv*psum_tf + b)
    Tables are therefore always HID=128 wide (256B rows) = the minimum
    dma_gather element size.
  - Gathers: batched `dma_gather` (SWDGE, mlp library) instead of
    per-tile indirect DMAs: ~1us fixed + ~0.34ns/desc, one call per
    (group of GRP dst blocks, table half).
  - int16 gather indices can't address 50176 rows, so each core's node
    rows are split into half A (blocks 0..24, 3200 rows) and half B
    (blocks 25..48, 3072 rows); AllGathers and gathers are per half.
    Half-A AllGather triggers as soon as blocks 0..24 have flushed,
    overlapping the rest of the layer.
  - Layer 1 needs no gathers: host pre-gathers x*dinv into edge-slot
    order (xgsw) with the same tile layout.

Sync-legality: this neuronxcc build allows at most ONE semaphore wait per
instruction; _legalize_waits spills extras onto same-engine NoOps.
"""

import numpy as np
import ml_dtypes

BF16 = ml_dtypes.bfloat16

# hardcoded problem shape (nn_GCNModel_68186900792261)
N = 50000
F_IN = 128
HID = 128
C = 40
NCORES = 8
P = 128
NBLK = 49          # blocks of 128 node slots per core
GRP = 4            # max dst blocks per gather/sel group
MAXKT = 30         # max tiles per dma_gather call (ring-capacity bound)
CC_LAG = 2         # groups to wait past half-A completion before AG trigger


def _derived():
    NPAD = NBLK * P
    ABLK = (NBLK + 1) // 2          # 25 blocks -> half A
    BBLK = NBLK - ABLK              # 24 blocks -> half B
    AR = ABLK * P                   # 3200 rows/core in half A
    BR = BBLK * P
    return NPAD, ABLK, BBLK, AR, BR


def _pack_nodes(deg):
    """Greedy degree-balanced packing of nodes into NCORES*NBLK bins of <=P."""
    import heapq

    nbins = NCORES * NBLK
    NPAD = NBLK * P
    order = np.argsort(-deg, kind="stable")
    heap = [(0, b) for b in range(nbins)]
    heapq.heapify(heap)
    nodecnt = np.zeros(nbins, np.int64)
    pos_of_node = np.empty(len(deg), np.int64)
    for n in order:
        while True:
            e, b = heapq.heappop(heap)
            if nodecnt[b] < P:
                break
        core, blk = divmod(b, NBLK)
        pos_of_node[n] = core * NPAD + blk * P + nodecnt[b]
        nodecnt[b] += 1
        heapq.heappush(heap, (e + int(deg[n]), b))
    return pos_of_node


def _wrap_idxs(arr):
    """arr [P, k] int16 (lane, tile) -> wrapped [128, k*8] for dma_gather:
    call idx i = t*128+lane lives at [i%16, i//16], tiled x8 over parts."""
    if arr.shape[1] == 0:
        return np.zeros((128, 0), np.int16)
    flat = arr.T.reshape(-1)                       # i = t*128 + lane
    w = flat.reshape(-1, 16).T                     # [16, k*8]
    return np.tile(w, (8, 1)).astype(np.int16)     # [128, k*8]


def _preprocess(x, W1, b1, W2, b2, W3, b3, edge_index):
    NPAD, ABLK, BBLK, AR, BR = _derived()
    NPADT = NCORES * NPAD

    src = np.asarray(edge_index[0], dtype=np.int64)
    dst = np.asarray(edge_index[1], dtype=np.int64)
    loop = np.arange(N, dtype=np.int64)
    src_all = np.concatenate([src, loop])
    dst_all = np.concatenate([dst, loop])

    deg = np.bincount(dst_all, minlength=N).astype(np.float64)
    dinv = 1.0 / np.sqrt(deg)

    pos = _pack_nodes(deg)

    spos = pos[src_all]
    dpos = pos[dst_all]
    sc = spos // NPAD
    sl = spos % NPAD
    shalf = (sl >= AR).astype(np.int64)
    srow = np.where(shalf == 0, sc * AR + sl, sc * BR + (sl - AR))
    ebin = dpos // P                     # global bin id (core*NBLK + j)
    rel = dpos % P

    key = ebin * 2 + shalf
    order = np.argsort(key, kind="stable")
    key_s = key[order]
    cnt = np.bincount(key_s, minlength=NCORES * NBLK * 2)
    seg_off = np.zeros_like(cnt)
    seg_off[1:] = np.cumsum(cnt)[:-1]
    rank = np.arange(len(key_s)) - seg_off[key_s]

    nAB = cnt.reshape(NCORES, NBLK, 2)
    TA = np.ceil(nAB[:, :, 0].max(axis=0) / P).astype(np.int64)   # [NBLK]
    TB = np.ceil(nAB[:, :, 1].max(axis=0) / P).astype(np.int64)

    # group structure + global tile order: per group, A tiles of its
    # slots then B tiles of its slots. Greedy grouping keeps each gather
    # call <= MAXKT tiles (SWDGE ring: num_idxs/16+1 descs/engine must
    # stay under the 256-desc carveout ring).
    groups = []
    g0 = 0
    while g0 < NBLK:
        g1 = g0 + 1
        ka, kb = int(TA[g0]), int(TB[g0])
        while (g1 < NBLK and g1 - g0 < GRP
               and ka + TA[g1] <= MAXKT and kb + TB[g1] <= MAXKT):
            ka += int(TA[g1])
            kb += int(TB[g1])
            g1 += 1
        groups.append((g0, g1))
        g0 = g1
    tile_off_A = np.zeros(NBLK, np.int64)
    tile_off_B = np.zeros(NBLK, np.int64)
    gmeta = []
    cur = 0
    for (g0, g1) in groups:
        gA0 = cur
        for j in range(g0, g1):
            tile_off_A[j] = cur
            cur += TA[j]
        kA = int(cur - gA0)
        gB0 = cur
        for j in range(g0, g1):
            tile_off_B[j] = cur
            cur += TB[j]
        kB = int(cur - gB0)
        gmeta.append(dict(g0=g0, g1=g1, gA0=int(gA0), kA=kA, gB0=int(gB0), kB=kB))
    GT2 = int(cur)

    c_e = key_s // (NBLK * 2)
    j_e = (key_s // 2) % NBLK
    h_e = key_s % 2
    tloc = rank // P
    lane = rank % P
    gidx = np.where(h_e == 0, tile_off_A[j_e], tile_off_B[j_e]) + tloc

    relf = np.full((NCORES, P, GT2), 255.0, np.float32)
    srcf = np.zeros((NCORES, P, GT2), np.int64)
    idxv = np.zeros((NCORES, P, GT2), np.int64)
    relf[c_e, lane, gidx] = rel[order].astype(np.float32)
    srcf[c_e, lane, gidx] = spos[order]
    idxv[c_e, lane, gidx] = srow[order]

    # layer-1 inputs pre-gathered in edge-slot order, dinv[src] folded in
    xpos = np.zeros((NPADT, F_IN), np.float32)
    xpos[pos] = np.asarray(x, np.float32) * dinv.astype(np.float32)[:, None]

    dinv_pos = np.zeros(NPADT, np.float32)
    dinv_pos[pos] = dinv.astype(np.float32)
    dinv_cols = dinv_pos.reshape(NCORES * NBLK, P).T.copy()

    iota = np.tile(np.arange(P, dtype=np.float32), (P, 1)).astype(BF16)

    colA_off, colB_off = [], []
    ca = cb = 0
    for gm in gmeta:
        colA_off.append(ca)
        colB_off.append(cb)
        ca += gm["kA"] * 8
        cb += gm["kB"] * 8

    cfg = dict(
        TA=TA.tolist(), TB=TB.tolist(), groups=groups, gmeta=gmeta,
        GT2=GT2, colsA=ca, colsB=cb, colA_off=colA_off, colB_off=colB_off,
        KAMAX=max(gm["kA"] for gm in gmeta),
        KBMAX=max(gm["kB"] for gm in gmeta),
        KTMAX=max(gm["kA"] + gm["kB"] for gm in gmeta),
    )

    common = {
        "W1": np.asarray(W1, np.float32).astype(BF16),
        "W2": np.asarray(W2, np.float32).astype(BF16),
        "W3": np.asarray(W3, np.float32).astype(BF16),
        "b1bc": np.tile(np.asarray(b1, np.float32).reshape(1, HID), (P, 1)),
        "b2bc": np.tile(np.asarray(b2, np.float32).reshape(1, HID), (P, 1)),
        "b3bc": np.tile(np.asarray(b3, np.float32).reshape(1, C), (P, 1)),
        "iota": iota,
    }
    in_maps = []
    for c in range(NCORES):
        m = dict(common)
        m["relflat"] = np.ascontiguousarray(relf[c]).astype(BF16)
        xg = xpos[srcf[c]]                         # [P, GT2, F]
        m["xgsw"] = np.ascontiguousarray(xg.reshape(P, GT2 * F_IN)).astype(BF16)
        iA = np.concatenate(
            [_wrap_idxs(idxv[c][:, gm["gA0"]: gm["gA0"] + gm["kA"]])
             for gm in gmeta], axis=1)
        iB = np.concatenate(
            [_wrap_idxs(idxv[c][:, gm["gB0"]: gm["gB0"] + gm["kB"]])
             for gm in gmeta], axis=1)
        m["idxA"] = np.ascontiguousarray(iA)
        m["idxB"] = np.ascontiguousarray(iB)
        m["dinv_cols_loc"] = np.ascontiguousarray(
            dinv_cols[:, c * NBLK: (c + 1) * NBLK])
        in_maps.append(m)
    return in_maps, pos, cfg


def _legalize_waits(nc, mybir, max_waits=1):
    """Spill excess sem waits onto same-engine NoOps (this neuronxcc build
    allows at most one wait per instruction)."""
    wn = 0
    for func in nc.m.functions:
        for bb in func.blocks:
            out = []
            changed = False
            for ins in bb.instructions:
                si = ins.sync_info
                if si is not None and si.on_wait and len(si.on_wait) > max_waits:
                    waits = list(si.on_wait)
                    for w in waits[:-max_waits]:
                        nop = mybir.InstNoOp(
                            name=f"WSPILL-{wn}",
                            engine=ins.engine,
                            sync_info=mybir.SyncInfo(on_wait=[w], on_update=[]),
                        )
                        wn += 1
                        out.append(nop)
                    ins.sync_info = mybir.SyncInfo(
                        on_wait=waits[-max_waits:], on_update=list(si.on_update)
                    )
                    changed = True
                out.append(ins)
            if changed:
                bb.instructions = out
    return wn


def _build_nc(cfg, legalize=True):
    import concourse.bass as bass
    import concourse.mybir as mybir
    import concourse.tile as tile
    from concourse import library_config
    from concourse.tile_rust import add_dep_helper

    NPAD, ABLK, BBLK, AR, BR = _derived()

    f32 = mybir.dt.float32
    bf16 = mybir.dt.bfloat16
    i16 = mybir.dt.int16
    GT2 = cfg["GT2"]
    gmeta = cfg["gmeta"]
    TA, TB = cfg["TA"], cfg["TB"]
    KAMAX, KBMAX, KTMAX = cfg["KAMAX"], cfg["KBMAX"], cfg["KTMAX"]

    nc = bass.Bass(target_bir_lowering=False, debug=False, num_devices=NCORES)

    xgsw = nc.dram_tensor("xgsw", [P, GT2 * F_IN], bf16, kind="ExternalInput")
    W1 = nc.dram_tensor("W1", [F_IN, HID], bf16, kind="ExternalInput")
    W2 = nc.dram_tensor("W2", [HID, HID], bf16, kind="ExternalInput")
    W3 = nc.dram_tensor("W3", [HID, C], bf16, kind="ExternalInput")
    b1bc = nc.dram_tensor("b1bc", [P, HID], f32, kind="ExternalInput")
    b2bc = nc.dram_tensor("b2bc", [P, HID], f32, kind="ExternalInput")
    b3bc = nc.dram_tensor("b3bc", [P, C], f32, kind="ExternalInput")
    iota = nc.dram_tensor("iota", [P, P], bf16, kind="ExternalInput")
    dinv_cols_loc = nc.dram_tensor("dinv_cols_loc", [P, NBLK], f32,
                                   kind="ExternalInput")
    relflat = nc.dram_tensor("relflat", [P, GT2], bf16, kind="ExternalInput")
    idxA = nc.dram_tensor("idxA", [P, cfg["colsA"]], i16, kind="ExternalInput")
    idxB = nc.dram_tensor("idxB", [P, cfg["colsB"]], i16, kind="ExternalInput")
    out = nc.dram_tensor("out", [NPAD, C], f32, kind="ExternalOutput")

    agin = {}
    tbl = {}
    for li in (2, 3):
        agin[(li, 0)] = nc.dram_tensor(f"agin{li}a", [AR, HID], bf16)
        agin[(li, 1)] = nc.dram_tensor(f"agin{li}b", [BR, HID], bf16)
        tbl[(li, 0)] = nc.dram_tensor(f"table{li}a", [NCORES * AR, HID], bf16,
                                      addr_space="Shared")
        tbl[(li, 1)] = nc.dram_tensor(f"table{li}b", [NCORES * BR, HID], bf16,
                                      addr_space="Shared")

    replica = [[i for i in range(NCORES)]]
    tail_deps = []

    def dep(later, earlier, sync=True, reason="gcn"):
        add_dep_helper(later.ins, earlier.ins, sync=sync, reason=reason)

    with tile.TileContext(nc) as tc:
        with (
            tc.tile_pool(name="const", bufs=1) as cpool,
            tc.tile_pool(name="xload", bufs=2) as xpool,
            tc.tile_pool(name="gata", bufs=2) as gApool,
            tc.tile_pool(name="gatb", bufs=2) as gBpool,
            tc.tile_pool(name="idx", bufs=3) as ipool,
            tc.tile_pool(name="sel", bufs=2) as spool,
            tc.tile_pool(name="aggt", bufs=3) as agpool,
            tc.tile_pool(name="flsh", bufs=4) as fpool,
            tc.tile_pool(name="wtbl", bufs=4) as wpool,
            tc.tile_pool(name="smax", bufs=4) as mpool,
            tc.tile_pool(name="psagg", bufs=4, space="PSUM") as apsum,
            tc.tile_pool(name="pstf", bufs=2, space="PSUM") as tpsum,
        ):
            nc.gpsimd.load_library(library_config.mlp)

            sb_W1 = cpool.tile([F_IN, HID], bf16, tag="w1")
            nc.sync.dma_start(out=sb_W1[:], in_=W1[:, :])
            sb_W2 = cpool.tile([HID, HID], bf16, tag="w2")
            nc.sync.dma_start(out=sb_W2[:], in_=W2[:, :])
            sb_W3 = cpool.tile([HID, C], bf16, tag="w3")
            nc.sync.dma_start(out=sb_W3[:], in_=W3[:, :])
            sb_b1 = cpool.tile([P, HID], f32, tag="b1")
            nc.sync.dma_start(out=sb_b1[:], in_=b1bc[:, :])
            sb_b2 = cpool.tile([P, HID], f32, tag="b2")
            nc.sync.dma_start(out=sb_b2[:], in_=b2bc[:, :])
            sb_b3 = cpool.tile([P, C], f32, tag="b3")
            nc.sync.dma_start(out=sb_b3[:], in_=b3bc[:, :])
            sb_iota = cpool.tile([P, P], bf16, tag="iota")
            nc.sync.dma_start(out=sb_iota[:], in_=iota[:, :])
            sb_dcols = cpool.tile([P, NBLK], f32, tag="dcols")
            nc.sync.dma_start(out=sb_dcols[:], in_=dinv_cols_loc[:, :])
            sb_rel = cpool.tile([P, GT2], bf16, tag="rel")
            nc.sync.dma_start(out=sb_rel[:], in_=relflat[:, :])

            # Pre-touch constants on their consuming engines.
            scr = cpool.tile([P, 8], f32, tag="scratch")
            nc.vector.tensor_copy(out=scr[:, 0:1], in_=sb_iota[:, 0:1])
            nc.vector.tensor_copy(out=scr[:, 1:2], in_=sb_rel[:, 0:1])
            nc.vector.tensor_copy(out=scr[:, 2:3], in_=sb_dcols[:, 0:1])
            nc.vector.tensor_copy(out=scr[:, 3:4], in_=sb_b1[:, 0:1])
            nc.vector.tensor_copy(out=scr[:, 4:5], in_=sb_b2[:, 0:1])
            nc.scalar.activation(
                out=scr[:, 5:6], in_=sb_b3[:, 0:1],
                func=mybir.ActivationFunctionType.Copy,
            )
            nc.scalar.activation(
                out=scr[:, 6:7], in_=sb_dcols[:, 0:1],
                func=mybir.ActivationFunctionType.Copy,
            )
            nc.tensor.ldweights(weights=sb_W1[:])
            nc.tensor.ldweights(weights=sb_W2[:])
            nc.tensor.ldweights(weights=sb_W3[:])

            # one gpsimd register per distinct gather num_idxs value
            nidx_regs = {}
            for gm in gmeta:
                for k in (gm["kA"], gm["kB"]):
                    if k * P not in nidx_regs:
                        nidx_regs[k * P] = nc.gpsimd.to_reg(k * P)

            def build_sel(gA0, ntiles):
                sel = spool.tile([P, KTMAX * P], bf16, tag="sel")
                sel3 = sel[:, : ntiles * P].rearrange("p (g s) -> p g s", s=P)
                nc.vector.tensor_tensor(
                    out=sel3,
                    in0=sb_iota[:, None, :].to_broadcast([P, ntiles, P]),
                    in1=sb_rel[:, gA0: gA0 + ntiles, None].to_broadcast(
                        [P, ntiles, P]),
                    op=mybir.AluOpType.is_equal,
                )
                return sel

            def emit_cc(half, writes, dst_agin, dst_tbl):
                last_ab = None
                for d in writes:
                    n = nc.gpsimd.nop()
                    dep(n, d, sync=True, reason=f"agin h{half} writes")
                    last_ab = n
                cc = nc.gpsimd.collective_compute(
                    "AllGather",
                    mybir.AluOpType.bypass,
                    replica_groups=replica,
                    ins=[dst_agin[half].ap().opt()],
                    outs=[dst_tbl[half].ap().opt()],
                )
                if last_ab is not None:
                    dep(cc, last_ab, sync=False, reason="cc after absorb")
                return cc

            def do_layer(li, Wsb, bias_sb, wout, src_tbl, dst_agin, dst_tbl):
                """One conv layer. src_tbl None => xgsw path;
                dst_agin None => final layer (softmax+out)."""
                halfw = {0: [], 1: []}
                ccs = []
                ccA_done = False
                ab_done_grp = None
                gathers = []
                last = []
                for gi, gm in enumerate(gmeta):
                    g0, g1, kA, kB = gm["g0"], gm["g1"], gm["kA"], gm["kB"]
                    ntiles = kA + kB
                    if src_tbl is None:
                        xg = xpool.tile([P, KTMAX * P], bf16, tag="xg")
                        nc.sync.dma_start(
                            out=xg[:, : ntiles * P],
                            in_=xgsw[:, gm["gA0"] * P: (gm["gA0"] + ntiles) * P],
                        )
                        srcsA = srcsB = xg
                        offB = kA
                    else:
                        assert kA > 0 and kB > 0
                        ixA = ipool.tile([P, KAMAX * 8], i16, tag="ixa")
                        nc.sync.dma_start(
                            out=ixA[:, : kA * 8],
                            in_=idxA[:, cfg["colA_off"][gi]:
                                     cfg["colA_off"][gi] + kA * 8],
                        )
                        gA = gApool.tile([P, KAMAX * P], bf16, tag="ga")
                        gathers.append(nc.gpsimd.dma_gather(
                            gA[:, : kA * P].rearrange("p (k f) -> p k f", f=HID),
                            src_tbl[0][:, :],
                            ixA[:, : kA * 8],
                            kA * P,
                            nidx_regs[kA * P],
                            HID,
                            single_packet=False,
                        ))
                        ixB = ipool.tile([P, KBMAX * 8], i16, tag="ixb")
                        nc.sync.dma_start(
                            out=ixB[:, : kB * 8],
                            in_=idxB[:, cfg["colB_off"][gi]:
                                     cfg["colB_off"][gi] + kB * 8],
                        )
                        gB = gBpool.tile([P, KBMAX * P], bf16, tag="gb")
                        gathers.append(nc.gpsimd.dma_gather(
                            gB[:, : kB * P].rearrange("p (k f) -> p k f", f=HID),
                            src_tbl[1][:, :],
                            ixB[:, : kB * 8],
                            kB * P,
                            nidx_regs[kB * P],
                            HID,
                            single_packet=False,
                        ))
                        srcsA, srcsB = gA, gB
                        offB = 0
                    sel = build_sel(gm["gA0"], ntiles)

                    aoff = 0
                    boff = 0
                    for j in range(g0, g1):
                        nT = TA[j] + TB[j]
                        ps = apsum.tile([P, P], f32, tag="psa")
                        ti = 0
                        for t in range(TA[j]):
                            lt = aoff + t
                            nc.tensor.matmul(
                                out=ps[:],
                                lhsT=srcsA[:, lt * P: (lt + 1) * P],
                                rhs=sel[:, lt * P: (lt + 1) * P],
                                start=(ti == 0),
                                stop=(ti == nT - 1),
                            )
                            ti += 1
                        for t in range(TB[j]):
                            st = kA + boff + t            # sel/tile order pos
                            bt = offB + boff + t          # pos within srcsB buf
                            nc.tensor.matmul(
                                out=ps[:],
                                lhsT=srcsB[:, bt * P: (bt + 1) * P],
                                rhs=sel[:, st * P: (st + 1) * P],
                                start=(ti == 0),
                                stop=(ti == nT - 1),
                            )
                            ti += 1
                        aoff += TA[j]
                        boff += TB[j]

                        aggT = agpool.tile([P, P], bf16, tag="aggT")
                        nc.vector.tensor_copy(out=aggT[:], in_=ps[:])
                        ps2f = tpsum.tile([P, HID], f32, tag="pst")
                        ps2 = ps2f[:, :wout]
                        mm2 = nc.tensor.matmul(
                            out=ps2, lhsT=aggT[:], rhs=Wsb[:],
                            start=True, stop=True,
                        )
                        u = fpool.tile([P, wout], f32, tag=f"u{li}")
                        nc.vector.tensor_scalar(
                            out=u[:], in0=ps2,
                            scalar1=sb_dcols[:, j: j + 1], scalar2=None,
                            op0=mybir.AluOpType.mult,
                        )
                        t_ = fpool.tile([P, wout], f32, tag=f"t{li}")
                        nc.vector.tensor_tensor(
                            out=t_[:], in0=u[:], in1=bias_sb[:],
                            op=mybir.AluOpType.add,
                        )
                        if dst_agin is not None:
                            w = wpool.tile([P, HID], bf16, tag="wtbl")
                            act = nc.scalar.activation(
                                out=w[:], in_=t_[:],
                                func=mybir.ActivationFunctionType.Relu,
                                scale=sb_dcols[:, j: j + 1],
                            )
                            if j < ABLK:
                                wd = nc.sync.dma_start(
                                    out=dst_agin[0][j * P: (j + 1) * P, :],
                                    in_=w[:],
                                )
                                halfw[0].append(wd)
                            else:
                                wd = nc.sync.dma_start(
                                    out=dst_agin[1][(j - ABLK) * P:
                                                    (j - ABLK + 1) * P, :],
                                    in_=w[:],
                                )
                                halfw[1].append(wd)
                            last = [mm2, act, wd]
                        else:
                            mx = mpool.tile([P, 1], f32, tag="mx")
                            nc.vector.tensor_reduce(
                                out=mx[:], in_=t_[:],
                                axis=mybir.AxisListType.X,
                                op=mybir.AluOpType.max,
                            )
                            nmx = mpool.tile([P, 1], f32, tag="nmx")
                            nc.vector.tensor_scalar(
                                out=nmx[:], in0=mx[:], scalar1=-1.0,
                                scalar2=None, op0=mybir.AluOpType.mult,
                            )
                            ex = mpool.tile([P, C], f32, tag="ex")
                            ssum = mpool.tile([P, 1], f32, tag="ssum")
                            nc.scalar.activation(
                                out=ex[:], in_=t_[:],
                                func=mybir.ActivationFunctionType.Exp,
                                bias=nmx[:, :1], accum_out=ssum[:, :1],
                            )
                            ls = mpool.tile([P, 1], f32, tag="ls")
                            act2 = nc.scalar.activation(
                                out=ls[:], in_=ssum[:],
                                func=mybir.ActivationFunctionType.Ln,
                            )
                            tot = mpool.tile([P, 1], f32, tag="tot")
                            nc.vector.tensor_tensor(
                                out=tot[:], in0=mx[:], in1=ls[:],
                                op=mybir.AluOpType.add,
                            )
                            fin = mpool.tile([P, C], f32, tag="fin")
                            nc.vector.tensor_scalar(
                                out=fin[:], in0=t_[:], scalar1=tot[:, :1],
                                scalar2=None, op0=mybir.AluOpType.subtract,
                            )
                            od = nc.sync.dma_start(
                                out=out[j * P: (j + 1) * P, :], in_=fin[:]
                            )
                            tail_deps.append(od)
                            last = [mm2, act2, od]
                        if j == ABLK - 1:
                            ab_done_grp = gi
                    if (dst_agin is not None and not ccA_done
                            and ab_done_grp is not None
                            and gi >= ab_done_grp + CC_LAG):
                        ccA_done = True
                        ccs.append(emit_cc(0, halfw[0], dst_agin, dst_tbl))
                if dst_agin is not None:
                    if not ccA_done:
                        ccs.append(emit_cc(0, halfw[0], dst_agin, dst_tbl))
                    ccs.append(emit_cc(1, halfw[1], dst_agin, dst_tbl))
                return ccs, gathers, last

            ccs2, _, _ = do_layer(
                1, sb_W1, sb_b1, HID, None,
                (agin[(2, 0)], agin[(2, 1)]), (tbl[(2, 0)], tbl[(2, 1)]))
            ccs3, g2, _ = do_layer(
                2, sb_W2, sb_b2, HID, (tbl[(2, 0)], tbl[(2, 1)]),
                (agin[(3, 0)], agin[(3, 1)]), (tbl[(3, 0)], tbl[(3, 1)]))
            _, g3, last3 = do_layer(
                3, sb_W3, sb_b3, C, (tbl[(3, 0)], tbl[(3, 1)]), None, None)

            for d in tail_deps + last3 + ccs2 + ccs3 + g3[-8:]:
                n = nc.sync.nop()
                dep(n, d, sync=True, reason="tail quiesce")

    if legalize:
        from concourse.library_overlay import lower_extended_insts

        lower_extended_insts(nc)
        _legalize_waits(nc, mybir)
    return nc


def _run(inputs, trace=False):
    import sys

    if "/opt/trn_rl_repo" not in sys.path:
        sys.path.insert(0, "/opt/trn_rl_repo")
    from concourse.bass_utils import run_bass_kernel_spmd

    in_maps, pos_of_node, cfg = _preprocess(**inputs)
    nc = _build_nc(cfg)
    res = run_bass_kernel_spmd(
        nc, in_maps, core_ids=list(range(NCORES)), trace=trace
    )
    outs = np.concatenate([res.results[c]["out"] for c in range(NCORES)], axis=0)
    full = outs[pos_of_node].astype(np.float32)
    return full, res


def kernel(**inputs):
    full, _ = _run(inputs, trace=False)
    return full


# revision 17
# speedup vs baseline: 1.3071x; 1.1187x over previous
"""GCN (3-layer, PyG GCNConv semantics) on 8 Trainium2 NeuronCores.

Strategy v2:
  - Nodes packed into 8*NBLK blocks of 128 slots, degree-balanced.
  - Conv is linear, so every layer aggregates FIRST (A @ table) and
    transforms after, per dst block:
        psum_agg[feat, dst]  = sum_tiles gathered_tile @ onehot_sel
        psum_tf [dst, fout]  = matmul(lhsT=aggT, rhs=W)
        table_next[dst, :]   = dinv * relu(dinv*psum_tf + b)
    Tables are therefore always HID=128 wide (256B rows) = the minimum
    dma_gather element size.
  - Gathers: batched `dma_gather` (SWDGE, mlp library) instead of
    per-tile indirect DMAs: ~1us fixed + ~0.34ns/desc, one call per
    (group of GRP dst blocks, table half).
  - int16 gather indices can't address 50176 rows, so each core's node
    rows are split into half A (blocks 0..24, 3200 rows) and half B
    (blocks 25..48, 3072 rows); AllGathers and gathers are per half.
    Half-A AllGather triggers as soon as blocks 0..24 have flushed,
    overlapping the rest of the layer.
  - Layer 1 needs no gathers: host pre-gathers x*dinv into edge-slot
    order (xgsw) with the same tile layout.

Sync-legality: this neuronxcc build allows at most ONE semaphore wait per
instruction; _legalize_waits spills extras onto same-engine NoOps.
"""

import numpy as np
import ml_dtypes

BF16 = ml_dtypes.bfloat16

# hardcoded problem shape (nn_GCNModel_68186900792261)
N = 50000
F_IN = 128
HID = 128
C = 40
NCORES = 8
P = 128
NBLK = 49          # blocks of 128 node slots per core
GRP = 4            # max dst blocks per gather/sel group
MAXKT = 24         # max tiles per indirect gather call (ring bound)
CC_LAG = 2         # groups to wait past half-A completion before AG trigger


def _derived():
    NPAD = NBLK * P
    ABLK = (NBLK + 1) // 2          # 25 blocks -> half A
    BBLK = NBLK - ABLK              # 24 blocks -> half B
    AR = ABLK * P                   # 3200 rows/core in half A
    BR = BBLK * P
    return NPAD, ABLK, BBLK, AR, BR


def _pack_nodes(deg):
    """Greedy degree-balanced packing of nodes into NCORES*NBLK bins of <=P."""
    import heapq

    nbins = NCORES * NBLK
    NPAD = NBLK * P
    order = np.argsort(-deg, kind="stable")
    heap = [(0, b) for b in range(nbins)]
    heapq.heapify(heap)
    nodecnt = np.zeros(nbins, np.int64)
    pos_of_node = np.empty(len(deg), np.int64)
    for n in order:
        while True:
            e, b = heapq.heappop(heap)
            if nodecnt[b] < P:
                break
        core, blk = divmod(b, NBLK)
        pos_of_node[n] = core * NPAD + blk * P + nodecnt[b]
        nodecnt[b] += 1
        heapq.heappush(heap, (e + int(deg[n]), b))
    return pos_of_node


def _wrap_idxs(arr):
    """arr [P, k] int16 (lane, tile) -> wrapped [128, k*8] for dma_gather:
    call idx i = t*128+lane lives at [i%16, i//16], tiled x8 over parts."""
    if arr.shape[1] == 0:
        return np.zeros((128, 0), np.int16)
    flat = arr.T.reshape(-1)                       # i = t*128 + lane
    w = flat.reshape(-1, 16).T                     # [16, k*8]
    return np.tile(w, (8, 1)).astype(np.int16)     # [128, k*8]


def _preprocess(x, W1, b1, W2, b2, W3, b3, edge_index):
    NPAD, ABLK, BBLK, AR, BR = _derived()
    NPADT = NCORES * NPAD

    src = np.asarray(edge_index[0], dtype=np.int64)
    dst = np.asarray(edge_index[1], dtype=np.int64)
    loop = np.arange(N, dtype=np.int64)
    src_all = np.concatenate([src, loop])
    dst_all = np.concatenate([dst, loop])

    deg = np.bincount(dst_all, minlength=N).astype(np.float64)
    dinv = 1.0 / np.sqrt(deg)

    pos = _pack_nodes(deg)

    spos = pos[src_all]
    dpos = pos[dst_all]
    sc = spos // NPAD
    sl = spos % NPAD
    shalf = (sl >= AR).astype(np.int64)
    srow = np.where(shalf == 0, sc * AR + sl, sc * BR + (sl - AR))
    ebin = dpos // P                     # global bin id (core*NBLK + j)
    rel = dpos % P

    key = ebin * 2 + shalf
    order = np.argsort(key, kind="stable")
    key_s = key[order]
    cnt = np.bincount(key_s, minlength=NCORES * NBLK * 2)
    seg_off = np.zeros_like(cnt)
    seg_off[1:] = np.cumsum(cnt)[:-1]
    rank = np.arange(len(key_s)) - seg_off[key_s]

    nAB = cnt.reshape(NCORES, NBLK, 2)
    TA = np.ceil(nAB[:, :, 0].max(axis=0) / P).astype(np.int64)   # [NBLK]
    TB = np.ceil(nAB[:, :, 1].max(axis=0) / P).astype(np.int64)

    # group structure + global tile order: per group, A tiles of its
    # slots then B tiles of its slots. Greedy grouping keeps each gather
    # call <= MAXKT tiles (SWDGE ring: num_idxs/16+1 descs/engine must
    # stay under the 256-desc carveout ring).
    groups = []
    g0 = 0
    while g0 < NBLK:
        g1 = g0 + 1
        ka, kb = int(TA[g0]), int(TB[g0])
        while (g1 < NBLK and g1 - g0 < GRP
               and ka + TA[g1] <= MAXKT and kb + TB[g1] <= MAXKT):
            ka += int(TA[g1])
            kb += int(TB[g1])
            g1 += 1
        groups.append((g0, g1))
        g0 = g1
    tile_off_A = np.zeros(NBLK, np.int64)
    tile_off_B = np.zeros(NBLK, np.int64)
    gmeta = []
    cur = 0
    for (g0, g1) in groups:
        gA0 = cur
        for j in range(g0, g1):
            tile_off_A[j] = cur
            cur += TA[j]
        kA = int(cur - gA0)
        gB0 = cur
        for j in range(g0, g1):
            tile_off_B[j] = cur
            cur += TB[j]
        kB = int(cur - gB0)
        gmeta.append(dict(g0=g0, g1=g1, gA0=int(gA0), kA=kA, gB0=int(gB0), kB=kB))
    GT2 = int(cur)

    c_e = key_s // (NBLK * 2)
    j_e = (key_s // 2) % NBLK
    h_e = key_s % 2
    tloc = rank // P
    lane = rank % P
    gidx = np.where(h_e == 0, tile_off_A[j_e], tile_off_B[j_e]) + tloc

    relf = np.full((NCORES, P, GT2), 255.0, np.float32)
    srcf = np.zeros((NCORES, P, GT2), np.int64)
    idxv = np.zeros((NCORES, P, GT2), np.int64)
    relf[c_e, lane, gidx] = rel[order].astype(np.float32)
    srcf[c_e, lane, gidx] = spos[order]
    idxv[c_e, lane, gidx] = srow[order]

    # layer-1 inputs pre-gathered in edge-slot order, dinv[src] folded in
    xpos = np.zeros((NPADT, F_IN), np.float32)
    xpos[pos] = np.asarray(x, np.float32) * dinv.astype(np.float32)[:, None]

    dinv_pos = np.zeros(NPADT, np.float32)
    dinv_pos[pos] = dinv.astype(np.float32)
    dinv_cols = dinv_pos.reshape(NCORES * NBLK, P).T.copy()

    iota = np.tile(np.arange(P, dtype=np.float32), (P, 1)).astype(BF16)

    colA_off, colB_off = [], []
    ca = cb = 0
    for gm in gmeta:
        colA_off.append(ca)
        colB_off.append(cb)
        ca += gm["kA"]
        cb += gm["kB"]

    cfg = dict(
        TA=TA.tolist(), TB=TB.tolist(), groups=groups, gmeta=gmeta,
        GT2=GT2, colsA=ca, colsB=cb, colA_off=colA_off, colB_off=colB_off,
        KAMAX=max(gm["kA"] for gm in gmeta),
        KBMAX=max(gm["kB"] for gm in gmeta),
        KTMAX=max(gm["kA"] + gm["kB"] for gm in gmeta),
    )

    common = {
        "W1": np.asarray(W1, np.float32).astype(BF16),
        "W2": np.asarray(W2, np.float32).astype(BF16),
        "W3": np.asarray(W3, np.float32).astype(BF16),
        "b1bc": np.tile(np.asarray(b1, np.float32).reshape(1, HID), (P, 1)),
        "b2bc": np.tile(np.asarray(b2, np.float32).reshape(1, HID), (P, 1)),
        "b3bc": np.tile(np.asarray(b3, np.float32).reshape(1, C), (P, 1)),
        "iota": iota,
    }
    in_maps = []
    for c in range(NCORES):
        m = dict(common)
        m["relflat"] = np.ascontiguousarray(relf[c]).astype(BF16)
        xg = xpos[srcf[c]]                         # [P, GT2, F]
        m["xgsw"] = np.ascontiguousarray(xg.reshape(P, GT2 * F_IN)).astype(BF16)
        iA = np.concatenate(
            [idxv[c][:, gm["gA0"]: gm["gA0"] + gm["kA"]] for gm in gmeta],
            axis=1)
        iB = np.concatenate(
            [idxv[c][:, gm["gB0"]: gm["gB0"] + gm["kB"]] for gm in gmeta],
            axis=1)
        m["idxA"] = np.concatenate(
            [_wrap_idxs(idxv[c][:, gm["gA0"]: gm["gA0"] + gm["kA"]])
             for gm in gmeta], axis=1)
        m["idxB"] = np.concatenate(
            [_wrap_idxs(idxv[c][:, gm["gB0"]: gm["gB0"] + gm["kB"]])
             for gm in gmeta], axis=1)
        m["dinv_cols_loc"] = np.ascontiguousarray(
            dinv_cols[:, c * NBLK: (c + 1) * NBLK])
        in_maps.append(m)
    return in_maps, pos, cfg


def _legalize_waits(nc, mybir, max_waits=1):
    """Spill excess sem waits onto same-engine NoOps (this neuronxcc build
    allows at most one wait per instruction)."""
    wn = 0
    for func in nc.m.functions:
        for bb in func.blocks:
            out = []
            changed = False
            for ins in bb.instructions:
                si = ins.sync_info
                if si is not None and si.on_wait and len(si.on_wait) > max_waits:
                    waits = list(si.on_wait)
                    for w in waits[:-max_waits]:
                        nop = mybir.InstNoOp(
                            name=f"WSPILL-{wn}",
                            engine=ins.engine,
                            sync_info=mybir.SyncInfo(on_wait=[w], on_update=[]),
                        )
                        wn += 1
                        out.append(nop)
                    ins.sync_info = mybir.SyncInfo(
                        on_wait=waits[-max_waits:], on_update=list(si.on_update)
                    )
                    changed = True
                out.append(ins)
            if changed:
                bb.instructions = out
    return wn


def _build_nc(cfg, legalize=True):
    import concourse.bass as bass
    import concourse.mybir as mybir
    import concourse.tile as tile
    from concourse import library_config
    from concourse.tile_rust import add_dep_helper

    NPAD, ABLK, BBLK, AR, BR = _derived()

    f32 = mybir.dt.float32
    bf16 = mybir.dt.bfloat16
    i16 = mybir.dt.int16
    GT2 = cfg["GT2"]
    gmeta = cfg["gmeta"]
    TA, TB = cfg["TA"], cfg["TB"]
    KAMAX, KBMAX, KTMAX = cfg["KAMAX"], cfg["KBMAX"], cfg["KTMAX"]

    nc = bass.Bass(target_bir_lowering=False, debug=False, num_devices=NCORES)

    xgsw = nc.dram_tensor("xgsw", [P, GT2 * F_IN], bf16, kind="ExternalInput")
    W1 = nc.dram_tensor("W1", [F_IN, HID], bf16, kind="ExternalInput")
    W2 = nc.dram_tensor("W2", [HID, HID], bf16, kind="ExternalInput")
    W3 = nc.dram_tensor("W3", [HID, C], bf16, kind="ExternalInput")
    b1bc = nc.dram_tensor("b1bc", [P, HID], f32, kind="ExternalInput")
    b2bc = nc.dram_tensor("b2bc", [P, HID], f32, kind="ExternalInput")
    b3bc = nc.dram_tensor("b3bc", [P, C], f32, kind="ExternalInput")
    iota = nc.dram_tensor("iota", [P, P], bf16, kind="ExternalInput")
    dinv_cols_loc = nc.dram_tensor("dinv_cols_loc", [P, NBLK], f32,
                                   kind="ExternalInput")
    relflat = nc.dram_tensor("relflat", [P, GT2], bf16, kind="ExternalInput")
    idxA = nc.dram_tensor("idxA", [P, cfg["colsA"] * 8], i16, kind="ExternalInput")
    idxB = nc.dram_tensor("idxB", [P, cfg["colsB"] * 8], i16, kind="ExternalInput")
    out = nc.dram_tensor("out", [NPAD, C], f32, kind="ExternalOutput")

    agin = {}
    tbl = {}
    tbln = {}
    for li in (2, 3):
        agin[(li, 0)] = nc.dram_tensor(f"agin{li}a", [AR, HID], bf16)
        agin[(li, 1)] = nc.dram_tensor(f"agin{li}b", [BR, HID], bf16)
        tbl[(li, 0)] = nc.dram_tensor(f"table{li}a", [NCORES * AR, HID], bf16,
                                      addr_space="Shared")
        tbl[(li, 1)] = nc.dram_tensor(f"table{li}b", [NCORES * BR, HID], bf16,
                                      addr_space="Shared")
        tbln[(li, 0)] = nc.dram_tensor(f"tbln{li}a", [NCORES * AR, HID], bf16)
        tbln[(li, 1)] = nc.dram_tensor(f"tbln{li}b", [NCORES * BR, HID], bf16)

    replica = [[i for i in range(NCORES)]]
    tail_deps = []

    def dep(later, earlier, sync=True, reason="gcn"):
        add_dep_helper(later.ins, earlier.ins, sync=sync, reason=reason)

    with tile.TileContext(nc) as tc:
        with (
            tc.tile_pool(name="const", bufs=1) as cpool,
            tc.tile_pool(name="xload", bufs=2) as xpool,
            tc.tile_pool(name="gata", bufs=2) as gApool,
            tc.tile_pool(name="gatb", bufs=2) as gBpool,
            tc.tile_pool(name="idx", bufs=3) as ipool,
            tc.tile_pool(name="sel", bufs=2) as spool,
            tc.tile_pool(name="aggt", bufs=3) as agpool,
            tc.tile_pool(name="flsh", bufs=4) as fpool,
            tc.tile_pool(name="wtbl", bufs=4) as wpool,
            tc.tile_pool(name="smax", bufs=4) as mpool,
            tc.tile_pool(name="psagg", bufs=4, space="PSUM") as apsum,
            tc.tile_pool(name="pstf", bufs=2, space="PSUM") as tpsum,
        ):
            sb_W1 = cpool.tile([F_IN, HID], bf16, tag="w1")
            nc.sync.dma_start(out=sb_W1[:], in_=W1[:, :])
            sb_W2 = cpool.tile([HID, HID], bf16, tag="w2")
            nc.sync.dma_start(out=sb_W2[:], in_=W2[:, :])
            sb_W3 = cpool.tile([HID, C], bf16, tag="w3")
            nc.sync.dma_start(out=sb_W3[:], in_=W3[:, :])
            sb_b1 = cpool.tile([P, HID], f32, tag="b1")
            nc.sync.dma_start(out=sb_b1[:], in_=b1bc[:, :])
            sb_b2 = cpool.tile([P, HID], f32, tag="b2")
            nc.sync.dma_start(out=sb_b2[:], in_=b2bc[:, :])
            sb_b3 = cpool.tile([P, C], f32, tag="b3")
            nc.sync.dma_start(out=sb_b3[:], in_=b3bc[:, :])
            sb_iota = cpool.tile([P, P], bf16, tag="iota")
            nc.sync.dma_start(out=sb_iota[:], in_=iota[:, :])
            sb_dcols = cpool.tile([P, NBLK], f32, tag="dcols")
            nc.sync.dma_start(out=sb_dcols[:], in_=dinv_cols_loc[:, :])
            sb_rel = cpool.tile([P, GT2], bf16, tag="rel")
            nc.sync.dma_start(out=sb_rel[:], in_=relflat[:, :])

            # Pre-touch constants on their consuming engines.
            scr = cpool.tile([P, 8], f32, tag="scratch")
            nc.vector.tensor_copy(out=scr[:, 0:1], in_=sb_iota[:, 0:1])
            nc.vector.tensor_copy(out=scr[:, 1:2], in_=sb_rel[:, 0:1])
            nc.vector.tensor_copy(out=scr[:, 2:3], in_=sb_dcols[:, 0:1])
            nc.vector.tensor_copy(out=scr[:, 3:4], in_=sb_b1[:, 0:1])
            nc.vector.tensor_copy(out=scr[:, 4:5], in_=sb_b2[:, 0:1])
            nc.scalar.activation(
                out=scr[:, 5:6], in_=sb_b3[:, 0:1],
                func=mybir.ActivationFunctionType.Copy,
            )
            nc.scalar.activation(
                out=scr[:, 6:7], in_=sb_dcols[:, 0:1],
                func=mybir.ActivationFunctionType.Copy,
            )
            nc.tensor.ldweights(weights=sb_W1[:])
            nc.tensor.ldweights(weights=sb_W2[:])
            nc.tensor.ldweights(weights=sb_W3[:])
            nc.gpsimd.load_library(library_config.mlp)
            nidx_regs = {}
            for gm in gmeta:
                for k in (gm["kA"], gm["kB"]):
                    if k * P not in nidx_regs:
                        nidx_regs[k * P] = nc.gpsimd.to_reg(k * P)


            def build_sel(gA0, ntiles):
                sel = spool.tile([P, KTMAX * P], bf16, tag="sel")
                sel3 = sel[:, : ntiles * P].rearrange("p (g s) -> p g s", s=P)
                nc.vector.tensor_tensor(
                    out=sel3,
                    in0=sb_iota[:, None, :].to_broadcast([P, ntiles, P]),
                    in1=sb_rel[:, gA0: gA0 + ntiles, None].to_broadcast(
                        [P, ntiles, P]),
                    op=mybir.AluOpType.is_equal,
                )
                return sel

            def emit_cc(half, writes, dst_agin, dst_tbl):
                last_ab = None
                for d in writes:
                    n = nc.gpsimd.nop()
                    dep(n, d, sync=True, reason=f"agin h{half} writes")
                    last_ab = n
                cc = nc.gpsimd.collective_compute(
                    "AllGather",
                    mybir.AluOpType.bypass,
                    replica_groups=replica,
                    ins=[dst_agin[half].ap().opt()],
                    outs=[dst_tbl[half].ap().opt()],
                )
                if last_ab is not None:
                    dep(cc, last_ab, sync=False, reason="cc after absorb")
                return cc

            def do_layer(li, Wsb, bias_sb, wout, src_tbl, dst_agin, dst_tbl, dst_tbln=None):
                """One conv layer. src_tbl None => xgsw path;
                dst_agin None => final layer (softmax+out)."""
                halfw = {0: [], 1: []}
                ccs = []
                ccA_done = False
                ab_done_grp = None
                gathers = []
                last = []
                for gi, gm in enumerate(gmeta):
                    g0, g1, kA, kB = gm["g0"], gm["g1"], gm["kA"], gm["kB"]
                    ntiles = kA + kB
                    if src_tbl is None:
                        xg = xpool.tile([P, KTMAX * P], bf16, tag="xg")
                        nc.sync.dma_start(
                            out=xg[:, : ntiles * P],
                            in_=xgsw[:, gm["gA0"] * P: (gm["gA0"] + ntiles) * P],
                        )
                        srcsA = srcsB = xg
                        offB = kA
                    else:
                        assert kA > 0 and kB > 0
                        ixA = ipool.tile([P, KAMAX * 8], i16, tag="ixa")
                        nc.sync.dma_start(
                            out=ixA[:, : kA * 8],
                            in_=idxA[:, cfg["colA_off"][gi] * 8:
                                     (cfg["colA_off"][gi] + kA) * 8],
                        )
                        gA = gApool.tile([P, KAMAX * P], bf16, tag="ga")
                        gathers.append(nc.gpsimd.dma_gather(
                            gA[:, : kA * P].rearrange("p (k f) -> p k f", f=HID),
                            src_tbl[0][:, :],
                            ixA[:, : kA * 8],
                            kA * P,
                            nidx_regs[kA * P],
                            HID,
                            single_packet=False,
                        ))
                        ixB = ipool.tile([P, KBMAX * 8], i16, tag="ixb")
                        nc.sync.dma_start(
                            out=ixB[:, : kB * 8],
                            in_=idxB[:, cfg["colB_off"][gi] * 8:
                                     (cfg["colB_off"][gi] + kB) * 8],
                        )
                        gB = gBpool.tile([P, KBMAX * P], bf16, tag="gb")
                        gathers.append(nc.gpsimd.dma_gather(
                            gB[:, : kB * P].rearrange("p (k f) -> p k f", f=HID),
                            src_tbl[1][:, :],
                            ixB[:, : kB * 8],
                            kB * P,
                            nidx_regs[kB * P],
                            HID,
                            single_packet=False,
                        ))
                        srcsA, srcsB = gA, gB
                        offB = 0
                    sel = build_sel(gm["gA0"], ntiles)

                    aoff = 0
                    boff = 0
                    for j in range(g0, g1):
                        nT = TA[j] + TB[j]
                        ps = apsum.tile([P, P], f32, tag="psa")
                        ti = 0
                        for t in range(TA[j]):
                            lt = aoff + t
                            nc.tensor.matmul(
                                out=ps[:],
                                lhsT=srcsA[:, lt * P: (lt + 1) * P],
                                rhs=sel[:, lt * P: (lt + 1) * P],
                                start=(ti == 0),
                                stop=(ti == nT - 1),
                            )
                            ti += 1
                        for t in range(TB[j]):
                            st = kA + boff + t            # sel/tile order pos
                            bt = offB + boff + t          # pos within srcsB buf
                            nc.tensor.matmul(
                                out=ps[:],
                                lhsT=srcsB[:, bt * P: (bt + 1) * P],
                                rhs=sel[:, st * P: (st + 1) * P],
                                start=(ti == 0),
                                stop=(ti == nT - 1),
                            )
                            ti += 1
                        aoff += TA[j]
                        boff += TB[j]

                        aggT = agpool.tile([P, P], bf16, tag="aggT")
                        nc.scalar.activation(
                            out=aggT[:], in_=ps[:],
                            func=mybir.ActivationFunctionType.Copy,
                        )
                        ps2f = tpsum.tile([P, HID], f32, tag="pst")
                        ps2 = ps2f[:, :wout]
                        mm2 = nc.tensor.matmul(
                            out=ps2, lhsT=aggT[:], rhs=Wsb[:],
                            start=True, stop=True,
                        )
                        u = fpool.tile([P, wout], f32, tag=f"u{li}")
                        nc.scalar.activation(
                            out=u[:], in_=ps2,
                            func=mybir.ActivationFunctionType.Copy,
                            scale=sb_dcols[:, j: j + 1],
                        )
                        t_ = fpool.tile([P, wout], f32, tag=f"t{li}")
                        nc.vector.tensor_tensor(
                            out=t_[:], in0=u[:], in1=bias_sb[:],
                            op=mybir.AluOpType.add,
                        )
                        if dst_agin is not None:
                            w = wpool.tile([P, HID], bf16, tag="wtbl")
                            act = nc.scalar.activation(
                                out=w[:], in_=t_[:],
                                func=mybir.ActivationFunctionType.Relu,
                                scale=sb_dcols[:, j: j + 1],
                            )
                            if j < ABLK:
                                wd = nc.sync.dma_start(
                                    out=dst_agin[0][j * P: (j + 1) * P, :],
                                    in_=w[:],
                                )
                                halfw[0].append(wd)
                            else:
                                wd = nc.sync.dma_start(
                                    out=dst_agin[1][(j - ABLK) * P:
                                                    (j - ABLK + 1) * P, :],
                                    in_=w[:],
                                )
                                halfw[1].append(wd)
                            last = [mm2, act, wd]
                        else:
                            mx = mpool.tile([P, 1], f32, tag="mx")
                            nc.vector.tensor_reduce(
                                out=mx[:], in_=t_[:],
                                axis=mybir.AxisListType.X,
                                op=mybir.AluOpType.max,
                            )
                            nmx = mpool.tile([P, 1], f32, tag="nmx")
                            nc.vector.tensor_scalar(
                                out=nmx[:], in0=mx[:], scalar1=-1.0,
                                scalar2=None, op0=mybir.AluOpType.mult,
                            )
                            ex = mpool.tile([P, C], f32, tag="ex")
                            ssum = mpool.tile([P, 1], f32, tag="ssum")
                            nc.scalar.activation(
                                out=ex[:], in_=t_[:],
                                func=mybir.ActivationFunctionType.Exp,
                                bias=nmx[:, :1], accum_out=ssum[:, :1],
                            )
                            ls = mpool.tile([P, 1], f32, tag="ls")
                            act2 = nc.scalar.activation(
                                out=ls[:], in_=ssum[:],
                                func=mybir.ActivationFunctionType.Ln,
                            )
                            tot = mpool.tile([P, 1], f32, tag="tot")
                            nc.vector.tensor_tensor(
                                out=tot[:], in0=mx[:], in1=ls[:],
                                op=mybir.AluOpType.add,
                            )
                            fin = mpool.tile([P, C], f32, tag="fin")
                            nc.vector.tensor_scalar(
                                out=fin[:], in0=t_[:], scalar1=tot[:, :1],
                                scalar2=None, op0=mybir.AluOpType.subtract,
                            )
                            od = nc.sync.dma_start(
                                out=out[j * P: (j + 1) * P, :], in_=fin[:]
                            )
                            tail_deps.append(od)
                            last = [mm2, act2, od]
                        if j == ABLK - 1:
                            ab_done_grp = gi
                    if (dst_agin is not None and not ccA_done
                            and ab_done_grp is not None
                            and gi >= ab_done_grp + CC_LAG):
                        ccA_done = True
                        ccs.append(emit_cc(0, halfw[0], dst_agin, dst_tbl))
                if dst_agin is not None:
                    if not ccA_done:
                        ccs.append(emit_cc(0, halfw[0], dst_agin, dst_tbl))
                    ccs.append(emit_cc(1, halfw[1], dst_agin, dst_tbl))
                return ccs, gathers, last

            ccs2, _, _ = do_layer(
                1, sb_W1, sb_b1, HID, None,
                (agin[(2, 0)], agin[(2, 1)]), (tbl[(2, 0)], tbl[(2, 1)]),
                (tbln[(2, 0)], tbln[(2, 1)]))
            ccs3, g2, _ = do_layer(
                2, sb_W2, sb_b2, HID, (tbl[(2, 0)], tbl[(2, 1)]),
                (agin[(3, 0)], agin[(3, 1)]), (tbl[(3, 0)], tbl[(3, 1)]),
                (tbln[(3, 0)], tbln[(3, 1)]))
            _, g3, last3 = do_layer(
                3, sb_W3, sb_b3, C, (tbl[(3, 0)], tbl[(3, 1)]), None, None)

            for d in tail_deps + last3 + ccs2 + ccs3 + g3[-8:]:
                n = nc.sync.nop()
                dep(n, d, sync=True, reason="tail quiesce")

    if legalize:
        from concourse.library_overlay import lower_extended_insts

        lower_extended_insts(nc)
        _legalize_waits(nc, mybir)
    return nc


def _run(inputs, trace=False):
    import sys

    if "/opt/trn_rl_repo" not in sys.path:
        sys.path.insert(0, "/opt/trn_rl_repo")
    from concourse.bass_utils import run_bass_kernel_spmd

    in_maps, pos_of_node, cfg = _preprocess(**inputs)
    nc = _build_nc(cfg)
    res = run_bass_kernel_spmd(
        nc, in_maps, core_ids=list(range(NCORES)), trace=trace
    )
    outs = np.concatenate([res.results[c]["out"] for c in range(NCORES)], axis=0)
    full = outs[pos_of_node].astype(np.float32)
    return full, res


def kernel(**inputs):
    full, _ = _run(inputs, trace=False)
    return full
